# revision 26
# baseline (speedup 1.0000x reference)
"""Black-Scholes 'all' pricing on 8 Trainium2 NeuronCores (Bass/Tile).

kernel(S0, K, T, vt) -> [N, 4] float32 (call, put, digital_call, digital_put)
N = 8_388_608; options sharded contiguously across 8 cores, each core
processing 1M elements as [128 partitions x 8192] in tiles of F=1024.

Design notes (each measured on HW; baseline 152us -> this kernel):
- GPSIMD runs NOTHING: DVE and GpSimd arbitrate for a shared SBUF port
  pair, lock-held for the full instruction - any GPS op fully blocks
  concurrent DVE two-source ops (measured: identical DVE TT ops swing
  678ns -> 2.6us under GPS load; removing GPS took exec 131us -> 110us).
- T/vt ship as f16; S0/K ship TWICE - f32 (feeds only the ACT ln) and
  f16 (feeds everything else). The extra 4 MiB/core of DMA is far under
  the DMA budget and turns the sq/kr muls all-f16 (DVE 2x). Digitals
  are untouched (their precision rides the f32 ln chain); call/put get
  ~5e-4 relative noise vs a 2e-2 tolerance.
- ln(S), ln(K) are taken with ACT scale=0.01 and written as float32r
  (PE matmul moving dtype). fp32r rounds to ~12 mantissa bits (measured
  rel 2.3e-4), so the scaling (ln(0.01*s) in [-0.23, 0.19] vs ~4.7
  unscaled) keeps abs error ~5e-5; ln(100) offsets cancel in lnS-lnK.
- numer = b + (R-Q)T + 0.5*vtt and numer2 = numer - vtt are computed ON
  THE PE ENGINE: identity-weight fp32r/f16 matmuls accumulating in PSUM
  (fp32r identity matmul measured bit-exact; mixed-dtype accumulation
  groups work; 16 matmuls + 5 weight loads per tile ~ 4us, PE is
  otherwise idle). d1 = numer*isv, d2 = numer2*isv read PSUM directly -
  PSUM operands use DVE's dedicated port (stable 1.22us mixed ops).
- Discount factors born f16: dq = exp(-Q t) on ACT, dr = dq^2 on DVE
  (R = 2Q). Output side all-f16 at DVE 2x / tensor_scalar 4x:
  sqkr = sk16*dqdr (wide pair), npair = 0.5*erf+0.5 (one wide 4x op),
  tpair = sqkr*npair, oc = t1-t2, pc = sq-kr, op = oc-pc,
  odc = dr*N2, odp = dr-odc. f16 TT measured 685ns per [128,1024].
- exp AND ln live in one ACT table set (natural_log_exp_and_others; its
  ln measured 1.9e-5 abs err, fine at 2e-2 tolerance), erf in
  sigmoid_and_others. Cross-group phase offset L(r)->E(r-1) merges
  adjacent L phases, measured only 4 ACT_TABLE_LOADs per core.
- DMA is packed: interleaved [s|k] f32, [s|k] f16, [t|v] f16 input pair
  planes and one [oc|op|odc|odp] f16 output plane per tile (4 DMAs/tile
  with 4-8KB per-partition lines). Identity-weight uploads are issued
  after the first tile's input DMAs so they don't delay the pipeline
  fill. Round sizes [2,3,2,1]: small drain round shortens the tail.
- Engine budget per core: DVE ~70us (the wall), ACT ~63us, DMA ~67us,
  PE ~30us, plus ~10us fill and ~8us drain/barrier.
"""
import numpy as np

import concourse.bass as bass
import concourse.tile as tile
from concourse import bacc, mybir
from concourse.bass_utils import run_bass_kernel_spmd
from concourse.tile_rust import add_dep_helper

F32 = mybir.dt.float32
F32R = mybir.dt.float32r
F16 = mybir.dt.float16
AF = mybir.ActivationFunctionType
OP = mybir.AluOpType

R = 0.02
Q = 0.01
INV_SQRT2 = 0.7071067811865476
SCL = 0.01  # ln input scale: ln(SCL*s), offsets cancel in b

N = 8_388_608
NCORES = 8
P = 128
FD = N // NCORES // P  # 8192

_KEEP_SETS = ("natural_log_exp_and_others", "sigmoid_and_others")
_orig_get_tables = None

_NC = None
LAST_EXEC_NS = None
LAST_TRACE_DIR = None
TRACE = False


def _patch_act_tables():
    """Blank the membership of every activation-table set except the two
    we use (list order preserved so act_func_set_id indices stay valid)
    so ln/exp resolve to the combined set and erf to sigmoid_and_others."""
    global _orig_get_tables
    import concourse.hw_specs as hw_specs
    if _orig_get_tables is None:
        _orig_get_tables = hw_specs.get_activation_tables

        def patched(arch):
            tabs = _orig_get_tables(arch)
            return {
                name: (fns if name in _KEEP_SETS else set())
                for name, fns in tabs.items()
            }

        hw_specs.get_activation_tables = patched
        bacc.get_activation_tables = patched


def build_bs(FD=FD, F=1024, G=3, P=P):
    from contextlib import ExitStack
    assert FD % F == 0
    _patch_act_tables()
    ntiles = FD // F
    nchunks = F // 512  # matmul moving-operand chunks (psum bank = 512 f32)
    nc = bacc.Bacc("TRN2", target_bir_lowering=False, debug=False,
                   num_devices=NCORES)
    sk_d = nc.dram_tensor("sk", [P, 2 * FD], F32, kind="ExternalInput").ap()
    skh_d = nc.dram_tensor("skh", [P, 2 * FD], F16, kind="ExternalInput").ap()
    tv_d = nc.dram_tensor("tv", [P, 2 * FD], F16, kind="ExternalInput").ap()
    eyeP_d = nc.dram_tensor("eyep", [P, P], F32R, kind="ExternalInput").ap()
    eyeN_d = nc.dram_tensor("eyen", [P, P], F32R, kind="ExternalInput").ap()
    eyeC_d = nc.dram_tensor("eyec", [P, P], F16, kind="ExternalInput").ap()
    eyeH_d = nc.dram_tensor("eyeh", [P, P], F16, kind="ExternalInput").ap()
    eyeH2_d = nc.dram_tensor("eyeh2", [P, P], F16, kind="ExternalInput").ap()
    out4_d = nc.dram_tensor("out4", [P, 4 * FD], F16, kind="ExternalOutput").ap()

    with tile.TileContext(nc) as tc, ExitStack() as ctx:
        inp = ctx.enter_context(tc.tile_pool(name="inp", bufs=3))
        inpsk = ctx.enter_context(tc.tile_pool(name="inpsk", bufs=2))
        inpskh = ctx.enter_context(tc.tile_pool(name="inpskh", bufs=2))
        consts = ctx.enter_context(tc.tile_pool(name="consts", bufs=1))
        mid1 = ctx.enter_context(tc.tile_pool(name="mid1", bufs=2 * G + 1))
        mid2 = ctx.enter_context(tc.tile_pool(name="mid2", bufs=2))
        mid3 = ctx.enter_context(tc.tile_pool(name="mid3", bufs=2))
        psA = ctx.enter_context(tc.tile_pool(name="psA", bufs=2, space="PSUM"))
        mid4 = ctx.enter_context(tc.tile_pool(name="mid4", bufs=2 * G + 1))
        mid5 = ctx.enter_context(tc.tile_pool(name="mid5", bufs=2))
        mid6 = ctx.enter_context(tc.tile_pool(name="mid6", bufs=2))
        outp = ctx.enter_context(tc.tile_pool(name="outp", bufs=2))

        eyes = []
        for name, dram, dt in (("eyeP", eyeP_d, F32R), ("eyeN", eyeN_d, F32R),
                               ("eyeC", eyeC_d, F16), ("eyeH", eyeH_d, F16),
                               ("eyeH2", eyeH2_d, F16)):
            eyes.append((consts.tile([P, P], dt, tag=name, name=name), dram))
        eyeP, eyeN, eyeC, eyeH, eyeH2 = (e[0] for e in eyes)

        sizes = [3, 4, 1]
        assert sum(sizes) == ntiles
        glist = []
        pos = 0
        for size in sizes:
            glist.append(range(pos, pos + size))
            pos += size

        prev_phase = []
        cur_phase = []

        def act(*args, **kwargs):
            bi = nc.scalar.activation(*args, **kwargs)
            for p in prev_phase:
                add_dep_helper(bi.ins, p.ins, sync=False,
                               reason="act table phase ordering")
            cur_phase.append(bi)
            return bi

        def end_phase():
            if cur_phase:
                prev_phase[:] = cur_phase
                cur_phase.clear()

        # Warmup: dependency-free 8-elem exp forces the ln/exp-set
        # ACT_TABLE_LOAD during the engine preamble / input-DMA window.
        warm = mid3.tile([P, 8], F32, tag="warm", bufs=1)
        nc.vector.memset(warm[:], 0.0)
        warm2 = mid3.tile([P, 8], F32, tag="warm2", bufs=1)
        act(warm2[:], warm[:], AF.Exp)

        st = {}  # per-tile tensor handles

        def emit_tail(tiles):
            # E-phase DVE tail + output DMA for a finished group
            for i in tiles:
                z = st.pop(i)
                npair = mid5.tile([P, 2, F], F16, tag="npair")
                nc.vector.tensor_scalar(npair[:], z["ep"][:], scalar1=0.5,
                                        scalar2=0.5, op0=OP.mult, op1=OP.add)
                tpair = mid6.tile([P, 2, F], F16, tag="tpair", bufs=1)
                nc.vector.tensor_mul(tpair[:], z["sqkr"][:], npair[:])
                out4 = outp.tile([P, 4, F], F16, tag="out4")
                nc.vector.tensor_sub(out4[:, 0], tpair[:, 0], tpair[:, 1])
                nc.vector.tensor_mul(out4[:, 2], z["dqdr"][:, 1], npair[:, 1])
                nc.vector.tensor_sub(out4[:, 1], out4[:, 0], z["pc"][:])
                nc.vector.tensor_sub(out4[:, 3], z["dqdr"][:, 1], out4[:, 2])
                nc.sync.dma_start(out4_d[:, i * 4 * F:(i + 1) * 4 * F],
                                  out4[:])

        prev_tiles = None
        for tiles in glist:
            # ---- L phase (ln/exp set) ----
            # vtt for the whole group goes FIRST on the DVE stream so the
            # ACT u-chain never stalls behind the previous round's tail
            # backlog (measured 4-7us ACT gaps otherwise).
            for i in tiles:
                tv = inp.tile([P, 2, F], F16, tag="tv")
                nc.sync.dma_start(tv[:], tv_d[:, i * 2 * F:(i + 1) * 2 * F])
                sk = inpsk.tile([P, 2, F], F32, tag="sk")
                nc.sync.dma_start(sk[:], sk_d[:, i * 2 * F:(i + 1) * 2 * F])
                skh = inpskh.tile([P, 2, F], F16, tag="skh")
                nc.sync.dma_start(skh[:], skh_d[:, i * 2 * F:(i + 1) * 2 * F])
                if i == 0:
                    # weight uploads ride behind the first tile's inputs so
                    # they don't delay the pipeline fill
                    for eye_t, eye_dram in eyes:
                        nc.sync.dma_start(eye_t[:], eye_dram)
                vtt = mid3.tile([P, F], F16, tag="vtt", bufs=G + 2)
                nc.vector.tensor_mul(vtt[:], tv[:, 0], tv[:, 1])
                st[i] = dict(tv=tv, sk=sk, skh=skh, vtt=vtt)
            # dq + lnsk first on ACT (u/isv for the group come after, by
            # which time the vtt's have drained)
            for i in tiles:
                z = st[i]
                dqdr = mid1.tile([P, 2, F], F16, tag="dqdr")
                act(dqdr[:, 0], z["tv"][:, 0], AF.Exp, scale=-Q)
                lnsk = mid2.tile([P, 2, F], F32R, tag="lnsk")
                act(lnsk[:], z["sk"][:], AF.Ln, scale=SCL)
                # dr = dq^2 on DVE (R = 2Q)
                nc.vector.tensor_mul(dqdr[:, 1], dqdr[:, 0], dqdr[:, 0])
                sqkr = mid4.tile([P, 2, F], F16, tag="sqkr")
                nc.vector.tensor_mul(sqkr[:], z["skh"][:], dqdr[:])
                pc = mid3.tile([P, F], F16, tag="pc", bufs=2 * G + 1)
                nc.vector.tensor_sub(pc[:], sqkr[:, 0], sqkr[:, 1])
                z.update(dqdr=dqdr, lnsk=lnsk, sqkr=sqkr, pc=pc)
            for i in tiles:
                z = st[i]
                u = mid3.tile([P, F], F16, tag="u")
                act(u[:], z["vtt"][:], AF.Ln)
                isv = mid3.tile([P, F], F16, tag="isv")
                act(isv[:], u[:], AF.Exp, scale=-0.5)
                z["isv"] = isv

                # numer / numer2 on PE: per-weight over both psum banks
                numer = psA.tile([P, F], F32, tag="numer")
                numer2 = psA.tile([P, F], F32, tag="numer2")
                for w, src, first, last in (
                    (eyeP, z["lnsk"][:, 0], True, False),
                    (eyeN, z["lnsk"][:, 1], False, False),
                    (eyeC, z["tv"][:, 0], False, False),
                    (eyeH, z["vtt"][:], False, True),
                    (eyeH2, z["vtt"][:], False, True),
                ):
                    banks = (numer, numer2) if w not in (eyeH, eyeH2) else \
                        ((numer,) if w is eyeH else (numer2,))
                    for bank in banks:
                        for c in range(nchunks):
                            cs = slice(c * 512, (c + 1) * 512)
                            nc.tensor.matmul(bank[:, cs], w[:], src[:, cs],
                                             start=first, stop=last,
                                             skip_group_check=True)

                dpair = mid4.tile([P, 2, F], F16, tag="dpair")
                nc.vector.tensor_mul(dpair[:, 0], numer[:], z["isv"][:])
                nc.vector.tensor_mul(dpair[:, 1], numer2[:], z["isv"][:])
                z["dpair"] = dpair
            end_phase()
            # ---- E phase (erf set) for the previous group ----
            if prev_tiles is not None:
                for i in prev_tiles:
                    z = st[i]
                    ep = mid6.tile([P, 2, F], F16, tag="ep")
                    act(ep[:], z["dpair"][:], AF.Erf, scale=INV_SQRT2)
                    z["ep"] = ep
                end_phase()
                emit_tail(prev_tiles)
            prev_tiles = tiles
        # drain the last group
        for i in prev_tiles:
            z = st[i]
            ep = mid6.tile([P, 2, F], F16, tag="ep")
            act(ep[:], z["dpair"][:], AF.Erf, scale=INV_SQRT2)
            z["ep"] = ep
        end_phase()
        emit_tail(prev_tiles)
    nc.compile()
    return nc


def _get_nc():
    global _NC
    if _NC is None:
        _NC = build_bs()
    return _NC


def kernel(S0, K, T, vt):
    global LAST_EXEC_NS, LAST_TRACE_DIR
    nc = _get_nc()
    F = 1024
    nt = FD // F
    s32 = np.asarray(S0, dtype=np.float32)
    k32 = np.asarray(K, dtype=np.float32)
    t16 = np.asarray(T, dtype=np.float32).astype(np.float16)
    v16 = np.asarray(vt, dtype=np.float32).astype(np.float16)
    eye = np.eye(P, dtype=np.float32)
    consts = {
        "eyep": eye, "eyen": -eye,
        "eyec": (eye * (R - Q)).astype(np.float16),
        "eyeh": (eye * 0.5).astype(np.float16),
        "eyeh2": (eye * -0.5).astype(np.float16),
    }
    shards = []
    for i in range(NCORES):
        sl = slice(i * P * FD, (i + 1) * P * FD)
        s_i = s32[sl].reshape(P, nt, F)
        k_i = k32[sl].reshape(P, nt, F)
        t_i = t16[sl].reshape(P, nt, F)
        v_i = v16[sl].reshape(P, nt, F)
        sk = np.stack([s_i, k_i], axis=2).reshape(P, 2 * FD)
        skh = sk.astype(np.float16)
        tv = np.stack([t_i, v_i], axis=2).reshape(P, 2 * FD)
        shards.append({"sk": np.ascontiguousarray(sk),
                       "skh": np.ascontiguousarray(skh),
                       "tv": np.ascontiguousarray(tv), **consts})
    kwargs = {}
    if TRACE:
        import tempfile
        LAST_TRACE_DIR = tempfile.mkdtemp(prefix="bs_trace_")
        kwargs = dict(trace=True, tmpdir=LAST_TRACE_DIR)
    res = run_bass_kernel_spmd(nc, shards, core_ids=list(range(NCORES)),
                               **kwargs)
    LAST_EXEC_NS = res.exec_time_ns
    out = np.empty((N, 4), dtype=np.float32)
    for i in range(NCORES):
        sl = slice(i * P * FD, (i + 1) * P * FD)
        o4 = res.results[i]["out4"].reshape(P, nt, 4, F)
        for c in range(4):
            out[sl, c] = o4[:, :, c, :].reshape(-1).astype(np.float32)
    return out


# revision 29
# speedup vs baseline: 1.0115x; 1.0115x over previous
"""Black-Scholes 'all' pricing on 8 Trainium2 NeuronCores (Bass/Tile).

kernel(S0, K, T, vt) -> [N, 4] float32 (call, put, digital_call, digital_put)
N = 8_388_608; options sharded contiguously across 8 cores, each core
processing 1M elements as [128 partitions x 8192] in tiles of F=1024.

Design notes (each measured on HW; baseline 152us -> this kernel):
- GPSIMD runs NOTHING: DVE and GpSimd arbitrate for a shared SBUF port
  pair, lock-held for the full instruction - any GPS op fully blocks
  concurrent DVE two-source ops (measured: identical DVE TT ops swing
  678ns -> 2.6us under GPS load; removing GPS took exec 131us -> 110us).
- T/vt ship as f16; S0/K ship TWICE - f32 (feeds only the ACT ln) and
  f16 (feeds everything else). The extra 4 MiB/core of DMA is far under
  the DMA budget and turns the sq/kr muls all-f16 (DVE 2x). Digitals
  are untouched (their precision rides the f32 ln chain); call/put get
  ~5e-4 relative noise vs a 2e-2 tolerance.
- ln(S), ln(K) are taken with ACT scale=0.01 and written as float32r
  (PE matmul moving dtype). fp32r rounds to ~12 mantissa bits (measured
  rel 2.3e-4), so the scaling (ln(0.01*s) in [-0.23, 0.19] vs ~4.7
  unscaled) keeps abs error ~5e-5; ln(100) offsets cancel in lnS-lnK.
- numer = b + (R-Q)T + 0.5*vtt and numer2 = numer - vtt are computed ON
  THE PE ENGINE: identity-weight fp32r/f16 matmuls accumulating in PSUM
  (fp32r identity matmul measured bit-exact; mixed-dtype accumulation
  groups work; 16 matmuls + 5 weight loads per tile ~ 4us, PE is
  otherwise idle). d1 = numer*isv, d2 = numer2*isv read PSUM directly -
  PSUM operands use DVE's dedicated port (stable 1.22us mixed ops).
- Discount factors born f16: dq = exp(-Q t) on ACT, dr = dq^2 on DVE
  (R = 2Q). Output side all-f16 at DVE 2x / tensor_scalar 4x:
  sqkr = sk16*dqdr (wide pair), npair = 0.5*erf+0.5 (one wide 4x op),
  tpair = sqkr*npair, oc = t1-t2, pc = sq-kr, op = oc-pc,
  odc = dr*N2, odp = dr-odc. f16 TT measured 685ns per [128,1024].
- exp AND ln live in one ACT table set (natural_log_exp_and_others; its
  ln measured 1.9e-5 abs err, fine at 2e-2 tolerance), erf in
  sigmoid_and_others. Cross-group phase offset L(r)->E(r-1) merges
  adjacent L phases, measured only 4 ACT_TABLE_LOADs per core.
- DMA is packed: interleaved [s|k] f32, [s|k] f16, [t|v] f16 input pair
  planes and one [oc|op|odc|odp] f16 output plane per tile (4 DMAs/tile
  with 4-8KB per-partition lines). Identity-weight uploads are issued
  after the first tile's input DMAs so they don't delay the pipeline
  fill. Round sizes [2,3,2,1]: small drain round shortens the tail.
- Engine budget per core: DVE ~70us (the wall), ACT ~63us, DMA ~67us,
  PE ~30us, plus ~10us fill and ~8us drain/barrier.
"""
import numpy as np

import concourse.bass as bass
import concourse.tile as tile
from concourse import bacc, mybir
from concourse.bass_utils import run_bass_kernel_spmd
from concourse.tile_rust import add_dep_helper

F32 = mybir.dt.float32
F32R = mybir.dt.float32r
F16 = mybir.dt.float16
AF = mybir.ActivationFunctionType
OP = mybir.AluOpType

R = 0.02
Q = 0.01
INV_SQRT2 = 0.7071067811865476
SCL = 0.01  # ln input scale: ln(SCL*s), offsets cancel in b

N = 8_388_608
NCORES = 8
P = 128
FD = N // NCORES // P  # 8192

_KEEP_SETS = ("natural_log_exp_and_others", "sigmoid_and_others")
_orig_get_tables = None

_NC = None
LAST_EXEC_NS = None
LAST_TRACE_DIR = None
TRACE = False


def _patch_act_tables():
    """Blank the membership of every activation-table set except the two
    we use (list order preserved so act_func_set_id indices stay valid)
    so ln/exp resolve to the combined set and erf to sigmoid_and_others."""
    global _orig_get_tables
    import concourse.hw_specs as hw_specs
    if _orig_get_tables is None:
        _orig_get_tables = hw_specs.get_activation_tables

        def patched(arch):
            tabs = _orig_get_tables(arch)
            return {
                name: (fns if name in _KEEP_SETS else set())
                for name, fns in tabs.items()
            }

        hw_specs.get_activation_tables = patched
        bacc.get_activation_tables = patched


def build_bs(FD=FD, F=1024, G=3, P=P):
    from contextlib import ExitStack
    assert FD % F == 0
    _patch_act_tables()
    ntiles = FD // F
    nchunks = F // 512  # matmul moving-operand chunks (psum bank = 512 f32)
    nc = bacc.Bacc("TRN2", target_bir_lowering=False, debug=False,
                   num_devices=NCORES)
    sk_d = nc.dram_tensor("sk", [P, 2 * FD], F32, kind="ExternalInput").ap()
    skh_d = nc.dram_tensor("skh", [P, 2 * FD], F16, kind="ExternalInput").ap()
    tv_d = nc.dram_tensor("tv", [P, 2 * FD], F16, kind="ExternalInput").ap()
    eyeP_d = nc.dram_tensor("eyep", [P, P], F32R, kind="ExternalInput").ap()
    eyeN_d = nc.dram_tensor("eyen", [P, P], F32R, kind="ExternalInput").ap()
    eyeC_d = nc.dram_tensor("eyec", [P, P], F16, kind="ExternalInput").ap()
    eyeH_d = nc.dram_tensor("eyeh", [P, P], F16, kind="ExternalInput").ap()
    eyeH2_d = nc.dram_tensor("eyeh2", [P, P], F16, kind="ExternalInput").ap()
    out4_d = nc.dram_tensor("out4", [P, 4 * FD], F16, kind="ExternalOutput").ap()

    with tile.TileContext(nc) as tc, ExitStack() as ctx:
        inp = ctx.enter_context(tc.tile_pool(name="inp", bufs=3))
        inpsk = ctx.enter_context(tc.tile_pool(name="inpsk", bufs=2))
        inpskh = ctx.enter_context(tc.tile_pool(name="inpskh", bufs=2))
        consts = ctx.enter_context(tc.tile_pool(name="consts", bufs=1))
        mid1 = ctx.enter_context(tc.tile_pool(name="mid1", bufs=2 * G + 1))
        mid2 = ctx.enter_context(tc.tile_pool(name="mid2", bufs=2))
        mid3 = ctx.enter_context(tc.tile_pool(name="mid3", bufs=2))
        psA = ctx.enter_context(tc.tile_pool(name="psA", bufs=2, space="PSUM"))
        mid4 = ctx.enter_context(tc.tile_pool(name="mid4", bufs=2 * G + 1))
        mid5 = ctx.enter_context(tc.tile_pool(name="mid5", bufs=2))
        mid6 = ctx.enter_context(tc.tile_pool(name="mid6", bufs=2))
        outp = ctx.enter_context(tc.tile_pool(name="outp", bufs=2))

        eyes = []
        for name, dram, dt in (("eyeP", eyeP_d, F32R), ("eyeN", eyeN_d, F32R),
                               ("eyeC", eyeC_d, F16), ("eyeH", eyeH_d, F16),
                               ("eyeH2", eyeH2_d, F16)):
            eyes.append((consts.tile([P, P], dt, tag=name, name=name), dram))
        eyeP, eyeN, eyeC, eyeH, eyeH2 = (e[0] for e in eyes)

        sizes = [2] + [G] * ((ntiles - 2) // G)
        assert sum(sizes) == ntiles
        glist = []
        pos = 0
        for size in sizes:
            glist.append(range(pos, pos + size))
            pos += size

        prev_phase = []
        cur_phase = []

        def act(*args, **kwargs):
            bi = nc.scalar.activation(*args, **kwargs)
            for p in prev_phase:
                add_dep_helper(bi.ins, p.ins, sync=False,
                               reason="act table phase ordering")
            cur_phase.append(bi)
            return bi

        def end_phase():
            if cur_phase:
                prev_phase[:] = cur_phase
                cur_phase.clear()

        # Warmup: dependency-free 8-elem exp forces the ln/exp-set
        # ACT_TABLE_LOAD during the engine preamble / input-DMA window.
        warm = mid3.tile([P, 8], F32, tag="warm", bufs=1)
        nc.vector.memset(warm[:], 0.0)
        warm2 = mid3.tile([P, 8], F32, tag="warm2", bufs=1)
        act(warm2[:], warm[:], AF.Exp)

        st = {}  # per-tile tensor handles

        def emit_tail(tiles):
            # E-phase DVE tail + output DMA for a finished group
            for i in tiles:
                z = st.pop(i)
                npair = mid5.tile([P, 2, F], F16, tag="npair")
                nc.vector.tensor_scalar(npair[:], z["ep"][:], scalar1=0.5,
                                        scalar2=0.5, op0=OP.mult, op1=OP.add)
                tpair = mid6.tile([P, 2, F], F16, tag="tpair")
                nc.vector.tensor_mul(tpair[:], z["sqkr"][:], npair[:])
                out4 = outp.tile([P, 4, F], F16, tag="out4")
                nc.vector.tensor_sub(out4[:, 0], tpair[:, 0], tpair[:, 1])
                nc.vector.tensor_mul(out4[:, 2], z["dqdr"][:, 1], npair[:, 1])
                nc.vector.tensor_sub(out4[:, 1], out4[:, 0], z["pc"][:])
                nc.vector.tensor_sub(out4[:, 3], z["dqdr"][:, 1], out4[:, 2])
                nc.sync.dma_start(out4_d[:, i * 4 * F:(i + 1) * 4 * F],
                                  out4[:])

        prev_tiles = None
        for tiles in glist:
            # ---- L phase (ln/exp set) ----
            for i in tiles:
                tv = inp.tile([P, 2, F], F16, tag="tv")
                nc.sync.dma_start(tv[:], tv_d[:, i * 2 * F:(i + 1) * 2 * F])
                sk = inpsk.tile([P, 2, F], F32, tag="sk")
                nc.sync.dma_start(sk[:], sk_d[:, i * 2 * F:(i + 1) * 2 * F])
                skh = inpskh.tile([P, 2, F], F16, tag="skh")
                nc.sync.dma_start(skh[:], skh_d[:, i * 2 * F:(i + 1) * 2 * F])
                if i == 0:
                    # weight uploads ride behind the first tile's inputs so
                    # they don't delay the pipeline fill
                    for eye_t, eye_dram in eyes:
                        nc.sync.dma_start(eye_t[:], eye_dram)

                vtt = mid3.tile([P, F], F16, tag="vtt", bufs=3)
                nc.vector.tensor_mul(vtt[:], tv[:, 0], tv[:, 1])
                dqdr = mid1.tile([P, 2, F], F16, tag="dqdr")
                act(dqdr[:, 0], tv[:, 0], AF.Exp, scale=-Q)
                lnsk = mid2.tile([P, 2, F], F32R, tag="lnsk")
                act(lnsk[:], sk[:], AF.Ln, scale=SCL)
                u = mid3.tile([P, F], F16, tag="u")
                act(u[:], vtt[:], AF.Ln)
                isv = mid3.tile([P, F], F16, tag="isv")
                act(isv[:], u[:], AF.Exp, scale=-0.5)
                # dr = dq^2 on DVE (R = 2Q)
                nc.vector.tensor_mul(dqdr[:, 1], dqdr[:, 0], dqdr[:, 0])

                # numer / numer2 on PE: per-weight over both psum banks
                numer = psA.tile([P, F], F32, tag="numer")
                numer2 = psA.tile([P, F], F32, tag="numer2")
                for w, src, first, last in (
                    (eyeP, lnsk[:, 0], True, False),
                    (eyeN, lnsk[:, 1], False, False),
                    (eyeC, tv[:, 0], False, False),
                    (eyeH, vtt[:], False, True),
                    (eyeH2, vtt[:], False, True),
                ):
                    banks = (numer, numer2) if w not in (eyeH, eyeH2) else \
                        ((numer,) if w is eyeH else (numer2,))
                    for bank in banks:
                        for c in range(nchunks):
                            cs = slice(c * 512, (c + 1) * 512)
                            nc.tensor.matmul(bank[:, cs], w[:], src[:, cs],
                                             start=first, stop=last,
                                             skip_group_check=True)

                # DVE mid-chain (all-f16 sqkr: skh is the f16 copy of s/k)
                sqkr = mid4.tile([P, 2, F], F16, tag="sqkr")
                nc.vector.tensor_mul(sqkr[:], skh[:], dqdr[:])
                pc = mid3.tile([P, F], F16, tag="pc", bufs=2 * G + 1)
                nc.vector.tensor_sub(pc[:], sqkr[:, 0], sqkr[:, 1])
                dpair = mid4.tile([P, 2, F], F16, tag="dpair")
                nc.vector.tensor_mul(dpair[:, 0], numer[:], isv[:])
                nc.vector.tensor_mul(dpair[:, 1], numer2[:], isv[:])
                st[i] = dict(dqdr=dqdr, sqkr=sqkr, pc=pc, dpair=dpair)
            end_phase()
            # ---- E phase (erf set) for the previous group ----
            if prev_tiles is not None:
                for i in prev_tiles:
                    z = st[i]
                    ep = mid6.tile([P, 2, F], F16, tag="ep")
                    act(ep[:], z["dpair"][:], AF.Erf, scale=INV_SQRT2)
                    z["ep"] = ep
                end_phase()
                emit_tail(prev_tiles)
            prev_tiles = tiles
        # drain the last group
        for i in prev_tiles:
            z = st[i]
            ep = mid6.tile([P, 2, F], F16, tag="ep")
            act(ep[:], z["dpair"][:], AF.Erf, scale=INV_SQRT2)
            z["ep"] = ep
        end_phase()
        emit_tail(prev_tiles)
    nc.compile()
    return nc


def _get_nc():
    global _NC
    if _NC is None:
        _NC = build_bs()
    return _NC


def kernel(S0, K, T, vt):
    global LAST_EXEC_NS, LAST_TRACE_DIR
    nc = _get_nc()
    F = 1024
    nt = FD // F
    s32 = np.asarray(S0, dtype=np.float32)
    k32 = np.asarray(K, dtype=np.float32)
    t16 = np.asarray(T, dtype=np.float32).astype(np.float16)
    v16 = np.asarray(vt, dtype=np.float32).astype(np.float16)
    eye = np.eye(P, dtype=np.float32)
    consts = {
        "eyep": eye, "eyen": -eye,
        "eyec": (eye * (R - Q)).astype(np.float16),
        "eyeh": (eye * 0.5).astype(np.float16),
        "eyeh2": (eye * -0.5).astype(np.float16),
    }
    shards = []
    for i in range(NCORES):
        sl = slice(i * P * FD, (i + 1) * P * FD)
        s_i = s32[sl].reshape(P, nt, F)
        k_i = k32[sl].reshape(P, nt, F)
        t_i = t16[sl].reshape(P, nt, F)
        v_i = v16[sl].reshape(P, nt, F)
        sk = np.stack([s_i, k_i], axis=2).reshape(P, 2 * FD)
        skh = sk.astype(np.float16)
        tv = np.stack([t_i, v_i], axis=2).reshape(P, 2 * FD)
        shards.append({"sk": np.ascontiguousarray(sk),
                       "skh": np.ascontiguousarray(skh),
                       "tv": np.ascontiguousarray(tv), **consts})
    kwargs = {}
    if TRACE:
        import tempfile
        LAST_TRACE_DIR = tempfile.mkdtemp(prefix="bs_trace_")
        kwargs = dict(trace=True, tmpdir=LAST_TRACE_DIR)
    res = run_bass_kernel_spmd(nc, shards, core_ids=list(range(NCORES)),
                               **kwargs)
    LAST_EXEC_NS = res.exec_time_ns
    out = np.empty((N, 4), dtype=np.float32)
    for i in range(NCORES):
        sl = slice(i * P * FD, (i + 1) * P * FD)
        o4 = res.results[i]["out4"].reshape(P, nt, 4, F)
        for c in range(4):
            out[sl, c] = o4[:, :, c, :].reshape(-1).astype(np.float32)
    return out
